# revision 34
# baseline (speedup 1.0000x reference)
"""Trainium2 Bass kernel for CosineAttention:

    out = sigmoid((xn @ xn.T) @ x)   where xn = x / ||x_row||

Reassociated: out = sigmoid(diag(sqrt r) . (y @ G)), y = diag(r^1/2) x,
G = y.T @ y  (symmetric, [D, D]); r = 1/||x_row||.  The O(N^2 D)
similarity matrix is never formed, and only ONE scaled fp16 copy of x
is needed (the row-scale sqrt(r) folds into the final sigmoid's
per-partition scale).  G's ~256 diagonal is fp16-safe, so no diagonal
shift machinery is needed.

Sharding: rows of x across 8 cores.  Each core computes the lower
triangle of partial G_c = y_c.T @ y_c (fp32 PSUM), AllReduces the
packed triangle in fp16, reconstructs mirrored blocks by PE transpose,
then computes its [N/8, D] output slice.

Collectives: meshes run serially in doorbell-firing order, and the
first mesh starts only ~25us after the last doorbell (all-ranks
rendezvous + ncfw wake; a pre-fired warmup mesh does NOT absorb this,
it only adds its own mesh time, so none is used).  Payload A = G rows
4..7 lower-tri (832KB, needed first by mm2) is computed and staged
strictly before payload B = G rows 0..3 tri (320KB, needed only by the
last quarter of mm2, so its mesh hides entirely under mm2 compute).
mm2 runs the column-half 512:1024 first (depends only on AR-A), in
PSUM groups of 4 banks so the mirror transposes interleave in the free
banks.  G loads stay off the gpsimd queue so the scheduler cannot
hoist them ahead of AR-B's doorbell.  yT and the G-block mirrors stay
on the PE: DMA-crossbar transposes (dma_start_transpose) measurably
corrupt data when any collective mesh or in-flight DMA shares the
XBAR, and cost ~1.25us of queue occupancy each.
"""

import numpy as np

import concourse.bass as bass  # noqa: F401
import concourse.mybir as mybir
import concourse.tile as tile
from concourse import bacc
from concourse.bass_utils import run_bass_kernel_spmd
from concourse.masks import make_identity

F32 = mybir.dt.float32
BF16 = mybir.dt.bfloat16
F16 = mybir.dt.float16
AFT = mybir.ActivationFunctionType

N, D = 8192, 1024
NCORES = 8
R = N // NCORES  # rows per core
P = 128
RT = R // P      # row tiles per core (8)
KT = D // P      # feature-block tiles (8)
FD = 512         # matmul moving free dim (one PSUM bank of f32)
GROUPS = [list(range(NCORES))]

# AR-A packed layout: [left parts mt=4..7 (4 x 512) | quad parts mt=4..7
# (128,256,384,512)]
A_LEFT = 4 * FD                              # 2048
QW = [(mt - 3) * P for mt in range(4, 8)]    # 128,256,384,512
QOFF = [A_LEFT]
for w in QW[:-1]:
    QOFF.append(QOFF[-1] + w)
TOT_A = A_LEFT + sum(QW)                     # 3328
# AR-B packed layout: rows 0..3 lower-tri widths
BW = [(mt + 1) * P for mt in range(4)]       # 128,256,384,512
BOFF = [0]
for w in BW[:-1]:
    BOFF.append(BOFF[-1] + w)
TOT_B = sum(BW)                              # 1280


def _emit_body(tc, xb, out, ctx):
    nc = tc.nc
    mm_dt = F16
    out_t = out.rearrange("(rt p) d -> rt p d", p=P)

    persist = ctx.enter_context(tc.tile_pool(name="persist", bufs=1))
    load = ctx.enter_context(tc.tile_pool(name="load", bufs=3))
    small = ctx.enter_context(tc.tile_pool(name="small", bufs=1))
    ostage = ctx.enter_context(tc.tile_pool(name="ostage", bufs=8))
    ps = ctx.enter_context(tc.tile_pool(name="ps", bufs=1, space="PSUM"))
    dram = ctx.enter_context(tc.tile_pool(name="dram", bufs=1, space="DRAM"))

    # ---- phase 0: staggered loads on three queues, norms, y ----
    xfall = persist.tile([P, RT, D], F32, tag="xfall")
    xb_r = xb.rearrange("(rt p) d -> p rt d", p=P)
    # tiny dummy sqrt so ACT loads its table while chunk 0 is in flight
    dumin = small.tile([P, 1], F32, tag="dumin")
    dumout = small.tile([P, 1], F32, tag="dumout")
    nc.vector.memset(dumin, 1.0)
    nc.scalar.sqrt(dumout, dumin)
    # tile 0 split in thirds across all three queues so it lands first
    nc.sync.dma_start(out=xfall[:, 0, 0:384], in_=xb_r[:, 0, 0:384])
    nc.gpsimd.dma_start(out=xfall[:, 0, 384:768], in_=xb_r[:, 0, 384:768])
    nc.scalar.dma_start(out=xfall[:, 0, 768:], in_=xb_r[:, 0, 768:])
    ENGS = [nc.sync, nc.gpsimd, nc.scalar, nc.sync, nc.gpsimd,
            nc.scalar, nc.sync]
    for rt in range(1, RT):
        ENGS[rt - 1].dma_start(out=xfall[:, rt, :], in_=xb_r[:, rt, :])

    ss_all = small.tile([P, RT], F32, tag="ss_all")
    iss_all = small.tile([P, RT], F32, tag="iss_all")
    r_all = small.tile([P, RT], F32, tag="r_all")
    sr_all = small.tile([P, RT], F32, tag="sr_all")
    y = []
    for rt in range(RT):
        xf = xfall[:, rt, :]
        sq = load.tile([P, D], BF16, tag="sq")
        nc.vector.affine_mul_reduce(sq, ss_all[:, rt:rt + 1], xf, xf,
                                    1.0, 0.0)
        nc.vector.reciprocal(iss_all[:, rt:rt + 1], ss_all[:, rt:rt + 1])
        nc.scalar.sqrt(r_all[:, rt:rt + 1], iss_all[:, rt:rt + 1])
        nc.scalar.sqrt(sr_all[:, rt:rt + 1], r_all[:, rt:rt + 1])
        t_y = persist.tile([P, D], mm_dt, tag=f"y{rt}", name=f"y{rt}")
        # y halves on GPSIMD + ACT, keeping the DVE queue free for the
        # reduces: a DVE y-mul gets queued behind the NEXT tiles'
        # data-blocked reduces (head-of-line), stalling mm1 ~6us
        nc.gpsimd.tensor_scalar_mul(t_y[:, 0:FD], xf[:, 0:FD],
                                    sr_all[:, rt:rt + 1])
        nc.scalar.activation(out=t_y[:, FD:], in_=xf[:, FD:], func=AFT.Copy,
                             scale=sr_all[:, rt:rt + 1])
        y.append(t_y)

    identb = persist.tile([P, P], mm_dt, tag="identb")
    make_identity(nc, identb)

    # ---- phase 1: mm1.  Pass 1 computes ALL of payload A: A-left
    # (rows 4..7 cols 0:512, banks 4..7) + quad-tri (rows 4..7 cols
    # 512.., banks 0..3); pass 2 computes payload B (rows 0..3 tri).
    # A must be staged strictly before B so A's doorbell fires first.
    psAL = [ps.tile([P, FD], F32, tag=f"acc{4 + i}", name=f"psAL{i}")
            for i in range(4)]
    psQ = [ps.tile([P, QW[i]], F32, tag=f"acc{i}", name=f"psQ{i}")
           for i in range(4)]
    for rt in range(RT):
        for i in range(4):
            nc.tensor.matmul(
                psAL[i],
                lhsT=y[rt][:, (4 + i) * P:(5 + i) * P],
                rhs=y[rt][:, 0:FD],
                start=(rt == 0), stop=(rt == RT - 1),
            )
            nc.tensor.matmul(
                psQ[i],
                lhsT=y[rt][:, (4 + i) * P:(5 + i) * P],
                rhs=y[rt][:, FD:FD + QW[i]],
                start=(rt == 0), stop=(rt == RT - 1),
            )

    # stage payload A on two queues, left parts first
    gA_s = persist.tile([P, TOT_A], mm_dt, tag="gA_s")
    a_in = dram.tile([P, TOT_A], mm_dt, tag="a_in")
    a_out = dram.tile([P, TOT_A], mm_dt, tag="a_out", addr_space="Shared")
    for i in range(4):
        if i % 2 == 0:
            nc.vector.tensor_copy(out=gA_s[:, i * FD:(i + 1) * FD],
                                  in_=psAL[i])
        else:
            nc.scalar.copy(out=gA_s[:, i * FD:(i + 1) * FD], in_=psAL[i])
    nc.sync.dma_start(out=a_in[:, 0:A_LEFT // 2], in_=gA_s[:, 0:A_LEFT // 2])
    nc.scalar.dma_start(out=a_in[:, A_LEFT // 2:A_LEFT],
                        in_=gA_s[:, A_LEFT // 2:A_LEFT])
    for i in range(4):
        if i % 2 == 0:
            nc.vector.tensor_copy(out=gA_s[:, QOFF[i]:QOFF[i] + QW[i]],
                                  in_=psQ[i])
        else:
            nc.scalar.copy(out=gA_s[:, QOFF[i]:QOFF[i] + QW[i]], in_=psQ[i])
    nc.sync.dma_start(out=a_in[:, A_LEFT:A_LEFT + 320],
                      in_=gA_s[:, A_LEFT:A_LEFT + 320])
    nc.scalar.dma_start(out=a_in[:, A_LEFT + 320:A_LEFT + 640],
                        in_=gA_s[:, A_LEFT + 320:A_LEFT + 640])
    nc.sync.dma_start(out=a_in[:, A_LEFT + 640:A_LEFT + 980],
                      in_=gA_s[:, A_LEFT + 640:A_LEFT + 980])
    nc.scalar.dma_start(out=a_in[:, A_LEFT + 980:],
                        in_=gA_s[:, A_LEFT + 980:])

    # pass 2: B-tri (banks 0..3, freed by the quad staging copies)
    psB = [ps.tile([P, BW[mt]], F32, tag=f"acc{mt}", name=f"psB{mt}")
           for mt in range(4)]
    for rt in range(RT):
        for mt in range(4):
            nc.tensor.matmul(
                psB[mt],
                lhsT=y[rt][:, mt * P:(mt + 1) * P],
                rhs=y[rt][:, 0:BW[mt]],
                start=(rt == 0), stop=(rt == RT - 1),
            )
    b_in = dram.tile([P, TOT_B], mm_dt, tag="b_in")
    b_out = dram.tile([P, TOT_B], mm_dt, tag="b_out", addr_space="Shared")
    gB_s = persist.tile([P, TOT_B], mm_dt, tag="gB_s")
    for mt in range(4):
        if mt % 2 == 0:
            nc.vector.tensor_copy(out=gB_s[:, BOFF[mt]:BOFF[mt] + BW[mt]],
                                  in_=psB[mt])
        else:
            nc.scalar.copy(out=gB_s[:, BOFF[mt]:BOFF[mt] + BW[mt]],
                           in_=psB[mt])
    # B staging on the two HWDGE queues (A's staging DMAs are already
    # drained by then), so trigB fires right after its copies land
    nc.sync.dma_start(out=b_in[:, 0:640], in_=gB_s[:, 0:640])
    nc.scalar.dma_start(out=b_in[:, 640:], in_=gB_s[:, 640:])

    # ---- collectives: AR-A first (needed first), AR-B second ----
    nc.gpsimd.collective_compute(
        "AllReduce", mybir.AluOpType.add, replica_groups=GROUPS,
        ins=[a_in.opt()], outs=[a_out.opt()],
    )
    nc.gpsimd.collective_compute(
        "AllReduce", mybir.AluOpType.add, replica_groups=GROUPS,
        ins=[b_in.opt()], outs=[b_out.opt()],
    )

    # ---- hidden in the AR window: yT transposes (4-bank rotation) ----
    yT = []
    for kt in range(KT):
        t_yT = persist.tile([P, D], mm_dt, tag=f"yT{kt}", name=f"yT{kt}")
        for rt in range(RT):
            src = y[rt][:, kt * P:(kt + 1) * P]
            tpt = ps.tile([P, P], mm_dt, tag=f"acc{4 + rt % 4}",
                          name=f"tp{kt}_{rt}")
            nc.tensor.transpose(tpt, src, identb)
            if rt % 2 == 0:
                nc.vector.tensor_copy(out=t_yT[:, rt * P:(rt + 1) * P],
                                      in_=tpt)
            else:
                nc.scalar.copy(out=t_yT[:, rt * P:(rt + 1) * P], in_=tpt)
        yT.append(t_yT)

    # preload the Sigmoid table while the mesh runs (input reads the
    # staged payload so the scheduler cannot hoist it into phase 0,
    # where it would thrash the ACT table against the Sqrts)
    dsin = small.tile([P, 1], F32, tag="dsin")
    nc.scalar.activation(out=dsin, in_=gA_s[:, 0:1], func=AFT.Sigmoid)

    # ---- G loads on sync/scalar ONLY (a gpsimd G-load can be hoisted
    # ahead of AR-B's doorbell on that queue, delaying the whole train)
    gr = [persist.tile([P, D], mm_dt, tag=f"gr{kt}", name=f"gr{kt}")
          for kt in range(KT)]
    # wave order is kt 7..4, and the early mirrors source only quad
    # regions, so load quads descending first, then the left parts
    # (first needed by the cross-mirrors a few us later)
    qe = [nc.sync, nc.scalar]
    for j, i in enumerate([3, 2, 1, 0]):
        qe[j % 2].dma_start(out=gr[4 + i][:, FD:FD + QW[i]],
                            in_=a_out[:, QOFF[i]:QOFF[i] + QW[i]])
    for j, i in enumerate([3, 2, 1, 0]):
        qe[j % 2].dma_start(out=gr[4 + i][:, 0:FD],
                            in_=a_out[:, i * FD:(i + 1) * FD])
    # B loads (fire when AR-B lands; gpsimd queue, after AR-B's trigger)
    for mt in range(4):
        nc.gpsimd.dma_start(out=gr[mt][:, 0:BW[mt]],
                            in_=b_out[:, BOFF[mt]:BOFF[mt] + BW[mt]])

    mir_n = [0]

    def mirror(dst_tile, dst_lo, src_tile, src_lo, slot, odd):
        mir_n[0] += 1
        tpu = ps.tile([P, P], mm_dt, tag=f"acc{slot}",
                      name=f"mir{mir_n[0]}")
        nc.tensor.transpose(tpu, src_tile[:, src_lo:src_lo + P], identb)
        if odd:
            nc.scalar.copy(out=dst_tile[:, dst_lo:dst_lo + P], in_=tpu)
        else:
            nc.vector.tensor_copy(out=dst_tile[:, dst_lo:dst_lo + P],
                                  in_=tpu)

    sig_n = [0]

    def drain(psz, mt, lo, last_half):
        # sigmoid with per-partition scale sqrt(r); fp16 out via SWDGE
        # (casts to f32), except f32 + HWDGE half-stores for the final
        # tiles to shorten the terminal drain.
        i = sig_n[0]
        sig_n[0] += 1
        if last_half and mt >= 6:
            # final two tiles: f32 sigmoid + half-stores on both HWDGE
            # queues to shorten the terminal drain
            ob32 = ostage.tile([P, FD], F32, tag="ob32")
            nc.scalar.activation(out=ob32, in_=psz, func=AFT.Sigmoid,
                                 scale=sr_all[:, mt:mt + 1])
            nc.sync.dma_start(out=out_t[mt][:, lo:lo + FD // 2],
                              in_=ob32[:, 0:FD // 2])
            nc.scalar.dma_start(out=out_t[mt][:, lo + FD // 2:lo + FD],
                                in_=ob32[:, FD // 2:])
        elif last_half and mt % 2 == 1:
            ob32 = ostage.tile([P, FD], F32, tag="ob32")
            nc.scalar.activation(out=ob32, in_=psz, func=AFT.Sigmoid,
                                 scale=sr_all[:, mt:mt + 1])
            nc.sync.dma_start(out=out_t[mt][:, lo:lo + FD], in_=ob32)
        else:
            ob = ostage.tile([P, FD], mm_dt, tag="ob")
            nc.scalar.activation(out=ob, in_=psz, func=AFT.Sigmoid,
                                 scale=sr_all[:, mt:mt + 1])
            nc.gpsimd.dma_start(out=out_t[mt][:, lo:lo + FD], in_=ob)

    def mm2_group(mts, banks, rhs_list, lo, last_half=False):
        psz = {}
        for j, mt in enumerate(mts):
            psz[mt] = ps.tile([P, FD], F32, tag=f"acc{banks[j]}",
                              name=f"psz{lo}_{mt}_{banks[j]}")
        for w, kt in enumerate([7, 6, 5, 4, 3, 2, 1]):
            for mt in mts:
                nc.tensor.matmul(
                    psz[mt],
                    lhsT=yT[kt][:, mt * P:(mt + 1) * P],
                    rhs=rhs_list[kt],
                    start=(w == 0), stop=False,
                )
        for mt in mts:
            nc.tensor.matmul(
                psz[mt],
                lhsT=yT[0][:, mt * P:(mt + 1) * P],
                rhs=rhs_list[0],
                start=False, stop=True,
            )
            drain(psz[mt], mt, lo, last_half)

    # group A (mt 0..3, banks 0..3) waves kt 7..4 start right away; the
    # 16 cross-mirrors (rows 0..3 cols 512:1024 = T of rows 4..7 cols
    # 0:512) run in banks 4..7 while group A holds 0..3.
    grX = [persist.tile([P, FD], mm_dt, tag=f"grX{r}", name=f"grX{r}")
           for r in range(4)]

    pszA = [ps.tile([P, FD], F32, tag=f"acc{j}", name=f"pszqA{j}")
            for j in range(4)]
    # A-mirrors (block (r, c), 4 <= r < c, into banks 4..7) interleave
    # with the kt 7..4 waves: wave kt needs only mirrors in column
    # blocks > kt of row kt, so wave 7 starts as soon as gr[7] loads
    n = 0
    AMIR = {7: [(6, 7)], 6: [(5, 6), (5, 7)], 5: [(4, 5), (4, 6), (4, 7)]}
    for w, kt in enumerate([7, 6, 5, 4]):
        for j, mt in enumerate(range(4)):
            nc.tensor.matmul(pszA[j],
                             lhsT=yT[kt][:, mt * P:(mt + 1) * P],
                             rhs=gr[kt][:, FD:],
                             start=(w == 0), stop=False)
        for r, c in AMIR.get(kt, []):
            mirror(gr[r], c * P, gr[c], r * P, 4 + n % 4, n % 2)
            n += 1
    n = 0
    for r in range(3, -1, -1):
        for c in range(4, 8):
            mirror(grX[r], (c - 4) * P, gr[c], r * P, 4 + n % 4, n % 2)
            n += 1
    for w, kt in enumerate([3, 2, 1, 0]):
        for j, mt in enumerate(range(4)):
            nc.tensor.matmul(pszA[j],
                             lhsT=yT[kt][:, mt * P:(mt + 1) * P],
                             rhs=grX[kt],
                             start=False, stop=(kt == 0))
            if kt == 0:
                drain(pszA[j], mt, FD, False)

    # group B (mt 4..7), banks 4..7
    rhsQ = [grX[0], grX[1], grX[2], grX[3],
            gr[4][:, FD:], gr[5][:, FD:], gr[6][:, FD:], gr[7][:, FD:]]
    mm2_group([4, 5, 6, 7], [4, 5, 6, 7], rhsQ, FD, last_half=False)

    # ---- mm2 half L: out cols 0:512 ----
    # B-mirrors: block (r, c), r < c <= 3 (banks 0..3 free after group A
    # of half Q drained)
    n = 0
    for r in range(4):
        for c in range(r + 1, 4):
            mirror(gr[r], c * P, gr[c], r * P, n % 4, n % 2)
            n += 1
    rhsL = [gr[kt][:, 0:FD] for kt in range(KT)]
    mm2_group([0, 1, 2, 3], [0, 1, 2, 3], rhsL, 0, last_half=False)
    mm2_group([4, 5, 6, 7], [4, 5, 6, 7], rhsL, 0, last_half=True)


def build():
    from contextlib import ExitStack

    nc = bacc.Bacc("TRN2", target_bir_lowering=False, debug=False,
                   num_devices=NCORES)
    xb = nc.dram_tensor("xb", [R, D], F32, kind="ExternalInput").ap()
    out = nc.dram_tensor("out", [R, D], F32, kind="ExternalOutput").ap()
    with tile.TileContext(nc) as tc:
        with ExitStack() as ctx:
            _emit_body(tc, xb, out, ctx)
    nc.compile()
    return nc


_NC_CACHE = {}


def _get_nc():
    if "nc" not in _NC_CACHE:
        _NC_CACHE["nc"] = build()
    return _NC_CACHE["nc"]


def kernel(x: np.ndarray) -> np.ndarray:
    x = np.asarray(x, dtype=np.float32)
    assert x.shape == (N, D), x.shape
    nc = _get_nc()
    in_maps = [{"xb": x[c * R:(c + 1) * R]} for c in range(NCORES)]
    res = run_bass_kernel_spmd(nc, in_maps, list(range(NCORES)))
    return np.concatenate([res.results[c]["out"] for c in range(NCORES)], axis=0)


# revision 36
# speedup vs baseline: 1.2870x; 1.2870x over previous
"""Trainium2 Bass kernel for CosineAttention:

    out = sigmoid((xn @ xn.T) @ x)   where xn = x / ||x_row||

Reassociated: out = sigmoid(diag(sqrt r) . (y @ G)), y = diag(r^1/2) x,
G = y.T @ y  (symmetric, [D, D]); r = 1/||x_row||.  The O(N^2 D)
similarity matrix is never formed, and only ONE scaled fp16 copy of x
is needed (the row-scale sqrt(r) folds into the final sigmoid's
per-partition scale).  G's ~256 diagonal is fp16-safe, so no diagonal
shift machinery is needed.

Sharding: rows of x across 8 cores.  Each core computes the lower
triangle of partial G_c = y_c.T @ y_c (fp32 PSUM), AllReduces the
packed triangle in fp16, reconstructs mirrored blocks by PE transpose,
then computes its [N/8, D] output slice.

Collectives: meshes run serially in doorbell-firing order, and the
first mesh starts only ~25us after the last doorbell (all-ranks
rendezvous + ncfw wake; a pre-fired warmup mesh does NOT absorb this,
it only adds its own mesh time, so none is used).  Payload A = G rows
4..7 lower-tri (832KB, needed first by mm2) is computed and staged
strictly before payload B = G rows 0..3 tri (320KB, needed only by the
last quarter of mm2, so its mesh hides entirely under mm2 compute).
mm2 runs the column-half 512:1024 first (depends only on AR-A), in
PSUM groups of 4 banks so the mirror transposes interleave in the free
banks.  G loads stay off the gpsimd queue so the scheduler cannot
hoist them ahead of AR-B's doorbell.  yT and the G-block mirrors stay
on the PE: DMA-crossbar transposes (dma_start_transpose) measurably
corrupt data when any collective mesh or in-flight DMA shares the
XBAR, and cost ~1.25us of queue occupancy each.
"""

import numpy as np

import concourse.bass as bass  # noqa: F401
import concourse.mybir as mybir
import concourse.tile as tile
from concourse import bacc
from concourse.bass_utils import run_bass_kernel_spmd
from concourse.masks import make_identity

F32 = mybir.dt.float32
BF16 = mybir.dt.bfloat16
F16 = mybir.dt.float16
AFT = mybir.ActivationFunctionType

N, D = 8192, 1024
NCORES = 8
R = N // NCORES  # rows per core
P = 128
RT = R // P      # row tiles per core (8)
KT = D // P      # feature-block tiles (8)
FD = 512         # matmul moving free dim (one PSUM bank of f32)
GROUPS = [list(range(NCORES))]

# AR-A packed layout: [left parts mt=4..7 (4 x 512) | quad parts mt=4..7
# (128,256,384,512)]
A_LEFT = 4 * FD                              # 2048
QW = [(mt - 3) * P for mt in range(4, 8)]    # 128,256,384,512
QOFF = [A_LEFT]
for w in QW[:-1]:
    QOFF.append(QOFF[-1] + w)
TOT_A = A_LEFT + sum(QW)                     # 3328
# AR-B packed layout: rows 0..3 lower-tri widths
BW = [(mt + 1) * P for mt in range(4)]       # 128,256,384,512
BOFF = [0]
for w in BW[:-1]:
    BOFF.append(BOFF[-1] + w)
TOT_B = sum(BW)                              # 1280


def _emit_body(tc, xb, out, ctx):
    nc = tc.nc
    mm_dt = F16
    out_t = out.rearrange("(rt p) d -> rt p d", p=P)

    persist = ctx.enter_context(tc.tile_pool(name="persist", bufs=1))
    load = ctx.enter_context(tc.tile_pool(name="load", bufs=3))
    small = ctx.enter_context(tc.tile_pool(name="small", bufs=1))
    ostage = ctx.enter_context(tc.tile_pool(name="ostage", bufs=8))
    ps = ctx.enter_context(tc.tile_pool(name="ps", bufs=1, space="PSUM"))
    dram = ctx.enter_context(tc.tile_pool(name="dram", bufs=1, space="DRAM"))

    # ---- phase 0: staggered loads on three queues, norms, y ----
    xfall = persist.tile([P, RT, D], F32, tag="xfall")
    xb_r = xb.rearrange("(rt p) d -> p rt d", p=P)
    # tiny dummy sqrt so ACT loads its table while chunk 0 is in flight
    dumin = small.tile([P, 1], F32, tag="dumin")
    dumout = small.tile([P, 1], F32, tag="dumout")
    nc.vector.memset(dumin, 1.0)
    nc.scalar.sqrt(dumout, dumin)
    # tile 0 split in thirds across all three queues so it lands first
    nc.sync.dma_start(out=xfall[:, 0, 0:384], in_=xb_r[:, 0, 0:384])
    nc.gpsimd.dma_start(out=xfall[:, 0, 384:768], in_=xb_r[:, 0, 384:768])
    nc.scalar.dma_start(out=xfall[:, 0, 768:], in_=xb_r[:, 0, 768:])
    ENGS = [nc.sync, nc.gpsimd, nc.scalar, nc.sync, nc.gpsimd,
            nc.scalar, nc.sync]
    for rt in range(1, RT):
        ENGS[rt - 1].dma_start(out=xfall[:, rt, :], in_=xb_r[:, rt, :])

    ss_all = small.tile([P, RT], F32, tag="ss_all")
    iss_all = small.tile([P, RT], F32, tag="iss_all")
    r_all = small.tile([P, RT], F32, tag="r_all")
    sr_all = small.tile([P, RT], F32, tag="sr_all")
    y = []
    for rt in range(RT):
        xf = xfall[:, rt, :]
        sq = load.tile([P, D], BF16, tag="sq")
        nc.vector.affine_mul_reduce(sq, ss_all[:, rt:rt + 1], xf, xf,
                                    1.0, 0.0)
        nc.vector.reciprocal(iss_all[:, rt:rt + 1], ss_all[:, rt:rt + 1])
        nc.scalar.sqrt(r_all[:, rt:rt + 1], iss_all[:, rt:rt + 1])
        nc.scalar.sqrt(sr_all[:, rt:rt + 1], r_all[:, rt:rt + 1])
        t_y = persist.tile([P, D], mm_dt, tag=f"y{rt}", name=f"y{rt}")
        # y entirely on ACT: a DVE y-mul gets queued behind the NEXT
        # tiles' data-blocked reduces on the Vector queue (head-of-line)
        # and stalls mm1 ~6us; ACT keeps per-tile chain order
        nc.scalar.activation(out=t_y, in_=xf, func=AFT.Copy,
                             scale=sr_all[:, rt:rt + 1])
        y.append(t_y)

    identb = persist.tile([P, P], mm_dt, tag="identb")
    make_identity(nc, identb)

    # ---- phase 1: mm1.  Pass 1 computes ALL of payload A: A-left
    # (rows 4..7 cols 0:512, banks 4..7) + quad-tri (rows 4..7 cols
    # 512.., banks 0..3); pass 2 computes payload B (rows 0..3 tri).
    # A must be staged strictly before B so A's doorbell fires first.
    psAL = [ps.tile([P, FD], F32, tag=f"acc{4 + i}", name=f"psAL{i}")
            for i in range(4)]
    psQ = [ps.tile([P, QW[i]], F32, tag=f"acc{i}", name=f"psQ{i}")
           for i in range(4)]
    for rt in range(RT):
        for i in range(4):
            nc.tensor.matmul(
                psAL[i],
                lhsT=y[rt][:, (4 + i) * P:(5 + i) * P],
                rhs=y[rt][:, 0:FD],
                start=(rt == 0), stop=(rt == RT - 1),
            )
            nc.tensor.matmul(
                psQ[i],
                lhsT=y[rt][:, (4 + i) * P:(5 + i) * P],
                rhs=y[rt][:, FD:FD + QW[i]],
                start=(rt == 0), stop=(rt == RT - 1),
            )

    # stage payload A on two queues, left parts first
    gA_s = persist.tile([P, TOT_A], mm_dt, tag="gA_s")
    a_in = dram.tile([P, TOT_A], mm_dt, tag="a_in")
    a_out = dram.tile([P, TOT_A], mm_dt, tag="a_out", addr_space="Shared")
    for i in range(4):
        if i % 2 == 0:
            nc.vector.tensor_copy(out=gA_s[:, i * FD:(i + 1) * FD],
                                  in_=psAL[i])
        else:
            nc.scalar.copy(out=gA_s[:, i * FD:(i + 1) * FD], in_=psAL[i])
    nc.sync.dma_start(out=a_in[:, 0:A_LEFT // 2], in_=gA_s[:, 0:A_LEFT // 2])
    nc.scalar.dma_start(out=a_in[:, A_LEFT // 2:A_LEFT],
                        in_=gA_s[:, A_LEFT // 2:A_LEFT])
    for i in range(4):
        if i % 2 == 0:
            nc.vector.tensor_copy(out=gA_s[:, QOFF[i]:QOFF[i] + QW[i]],
                                  in_=psQ[i])
        else:
            nc.scalar.copy(out=gA_s[:, QOFF[i]:QOFF[i] + QW[i]], in_=psQ[i])
    nc.sync.dma_start(out=a_in[:, A_LEFT:A_LEFT + 320],
                      in_=gA_s[:, A_LEFT:A_LEFT + 320])
    nc.scalar.dma_start(out=a_in[:, A_LEFT + 320:A_LEFT + 640],
                        in_=gA_s[:, A_LEFT + 320:A_LEFT + 640])
    nc.sync.dma_start(out=a_in[:, A_LEFT + 640:A_LEFT + 980],
                      in_=gA_s[:, A_LEFT + 640:A_LEFT + 980])
    nc.scalar.dma_start(out=a_in[:, A_LEFT + 980:],
                        in_=gA_s[:, A_LEFT + 980:])

    # pass 2: B-tri (banks 0..3, freed by the quad staging copies)
    psB = [ps.tile([P, BW[mt]], F32, tag=f"acc{mt}", name=f"psB{mt}")
           for mt in range(4)]
    for rt in range(RT):
        for mt in range(4):
            nc.tensor.matmul(
                psB[mt],
                lhsT=y[rt][:, mt * P:(mt + 1) * P],
                rhs=y[rt][:, 0:BW[mt]],
                start=(rt == 0), stop=(rt == RT - 1),
            )
    b_in = dram.tile([P, TOT_B], mm_dt, tag="b_in")
    b_out = dram.tile([P, TOT_B], mm_dt, tag="b_out", addr_space="Shared")
    gB_s = persist.tile([P, TOT_B], mm_dt, tag="gB_s")
    for mt in range(4):
        if mt % 2 == 0:
            nc.vector.tensor_copy(out=gB_s[:, BOFF[mt]:BOFF[mt] + BW[mt]],
                                  in_=psB[mt])
        else:
            nc.scalar.copy(out=gB_s[:, BOFF[mt]:BOFF[mt] + BW[mt]],
                           in_=psB[mt])
    # B staging on the two HWDGE queues (A's staging DMAs are already
    # drained by then), so trigB fires right after its copies land
    nc.sync.dma_start(out=b_in[:, 0:640], in_=gB_s[:, 0:640])
    nc.scalar.dma_start(out=b_in[:, 640:], in_=gB_s[:, 640:])

    # ---- collectives: AR-A first (needed first), AR-B second ----
    nc.gpsimd.collective_compute(
        "AllReduce", mybir.AluOpType.add, replica_groups=GROUPS,
        ins=[a_in.opt()], outs=[a_out.opt()],
    )
    nc.gpsimd.collective_compute(
        "AllReduce", mybir.AluOpType.add, replica_groups=GROUPS,
        ins=[b_in.opt()], outs=[b_out.opt()],
    )

    # ---- hidden in the AR window: yT transposes (4-bank rotation) ----
    yT = []
    for kt in range(KT):
        t_yT = persist.tile([P, D], mm_dt, tag=f"yT{kt}", name=f"yT{kt}")
        for rt in range(RT):
            src = y[rt][:, kt * P:(kt + 1) * P]
            tpt = ps.tile([P, P], mm_dt, tag=f"acc{4 + rt % 4}",
                          name=f"tp{kt}_{rt}")
            nc.tensor.transpose(tpt, src, identb)
            if rt % 2 == 0:
                nc.vector.tensor_copy(out=t_yT[:, rt * P:(rt + 1) * P],
                                      in_=tpt)
            else:
                nc.scalar.copy(out=t_yT[:, rt * P:(rt + 1) * P], in_=tpt)
        yT.append(t_yT)

    # preload the Sigmoid table while the mesh runs (input reads the
    # staged payload so the scheduler cannot hoist it into phase 0,
    # where it would thrash the ACT table against the Sqrts)
    dsin = small.tile([P, 1], F32, tag="dsin")
    nc.scalar.activation(out=dsin, in_=gA_s[:, 0:1], func=AFT.Sigmoid)

    # ---- G loads on sync/scalar ONLY (a gpsimd G-load can be hoisted
    # ahead of AR-B's doorbell on that queue, delaying the whole train)
    gr = [persist.tile([P, D], mm_dt, tag=f"gr{kt}", name=f"gr{kt}")
          for kt in range(KT)]
    # wave order is kt 7..4, and the early mirrors source only quad
    # regions, so load quads descending first, then the left parts
    # (first needed by the cross-mirrors a few us later)
    qe = [nc.sync, nc.scalar]
    for j, i in enumerate([3, 2, 1, 0]):
        qe[j % 2].dma_start(out=gr[4 + i][:, FD:FD + QW[i]],
                            in_=a_out[:, QOFF[i]:QOFF[i] + QW[i]])
    for j, i in enumerate([3, 2, 1, 0]):
        qe[j % 2].dma_start(out=gr[4 + i][:, 0:FD],
                            in_=a_out[:, i * FD:(i + 1) * FD])
    # B loads (fire when AR-B lands; gpsimd queue, after AR-B's trigger)
    for mt in range(4):
        nc.gpsimd.dma_start(out=gr[mt][:, 0:BW[mt]],
                            in_=b_out[:, BOFF[mt]:BOFF[mt] + BW[mt]])

    mir_n = [0]

    def mirror(dst_tile, dst_lo, src_tile, src_lo, slot, odd):
        mir_n[0] += 1
        tpu = ps.tile([P, P], mm_dt, tag=f"acc{slot}",
                      name=f"mir{mir_n[0]}")
        nc.tensor.transpose(tpu, src_tile[:, src_lo:src_lo + P], identb)
        if odd:
            nc.scalar.copy(out=dst_tile[:, dst_lo:dst_lo + P], in_=tpu)
        else:
            nc.vector.tensor_copy(out=dst_tile[:, dst_lo:dst_lo + P],
                                  in_=tpu)

    sig_n = [0]

    def drain(psz, mt, lo, last_half):
        # sigmoid with per-partition scale sqrt(r); fp16 out via SWDGE
        # (casts to f32), except f32 + HWDGE half-stores for the final
        # tiles to shorten the terminal drain.
        i = sig_n[0]
        sig_n[0] += 1
        if last_half and mt >= 6:
            # final two tiles: f32 sigmoid + half-stores on both HWDGE
            # queues to shorten the terminal drain
            ob32 = ostage.tile([P, FD], F32, tag="ob32")
            nc.scalar.activation(out=ob32, in_=psz, func=AFT.Sigmoid,
                                 scale=sr_all[:, mt:mt + 1])
            nc.sync.dma_start(out=out_t[mt][:, lo:lo + FD // 2],
                              in_=ob32[:, 0:FD // 2])
            nc.scalar.dma_start(out=out_t[mt][:, lo + FD // 2:lo + FD],
                                in_=ob32[:, FD // 2:])
        elif last_half and mt % 2 == 1:
            ob32 = ostage.tile([P, FD], F32, tag="ob32")
            nc.scalar.activation(out=ob32, in_=psz, func=AFT.Sigmoid,
                                 scale=sr_all[:, mt:mt + 1])
            nc.sync.dma_start(out=out_t[mt][:, lo:lo + FD], in_=ob32)
        else:
            ob = ostage.tile([P, FD], mm_dt, tag="ob")
            nc.scalar.activation(out=ob, in_=psz, func=AFT.Sigmoid,
                                 scale=sr_all[:, mt:mt + 1])
            nc.gpsimd.dma_start(out=out_t[mt][:, lo:lo + FD], in_=ob)

    def mm2_group(mts, banks, rhs_list, lo, last_half=False):
        psz = {}
        for j, mt in enumerate(mts):
            psz[mt] = ps.tile([P, FD], F32, tag=f"acc{banks[j]}",
                              name=f"psz{lo}_{mt}_{banks[j]}")
        for w, kt in enumerate([7, 6, 5, 4, 3, 2, 1]):
            for mt in mts:
                nc.tensor.matmul(
                    psz[mt],
                    lhsT=yT[kt][:, mt * P:(mt + 1) * P],
                    rhs=rhs_list[kt],
                    start=(w == 0), stop=False,
                )
        for mt in mts:
            nc.tensor.matmul(
                psz[mt],
                lhsT=yT[0][:, mt * P:(mt + 1) * P],
                rhs=rhs_list[0],
                start=False, stop=True,
            )
            drain(psz[mt], mt, lo, last_half)

    # group A (mt 0..3, banks 0..3) waves kt 7..4 start right away; the
    # 16 cross-mirrors (rows 0..3 cols 512:1024 = T of rows 4..7 cols
    # 0:512) run in banks 4..7 while group A holds 0..3.
    grX = [persist.tile([P, FD], mm_dt, tag=f"grX{r}", name=f"grX{r}")
           for r in range(4)]

    pszA = [ps.tile([P, FD], F32, tag=f"acc{j}", name=f"pszqA{j}")
            for j in range(4)]
    # A-mirrors (block (r, c), 4 <= r < c, into banks 4..7) interleave
    # with the kt 7..4 waves: wave kt needs only mirrors in column
    # blocks > kt of row kt, so wave 7 starts as soon as gr[7] loads
    n = 0
    AMIR = {7: [(6, 7)], 6: [(5, 6), (5, 7)], 5: [(4, 5), (4, 6), (4, 7)]}
    for w, kt in enumerate([7, 6, 5, 4]):
        for j, mt in enumerate(range(4)):
            nc.tensor.matmul(pszA[j],
                             lhsT=yT[kt][:, mt * P:(mt + 1) * P],
                             rhs=gr[kt][:, FD:],
                             start=(w == 0), stop=False)
        for r, c in AMIR.get(kt, []):
            mirror(gr[r], c * P, gr[c], r * P, 4 + n % 4, n % 2)
            n += 1
    n = 0
    for r in range(3, -1, -1):
        for c in range(4, 8):
            mirror(grX[r], (c - 4) * P, gr[c], r * P, 4 + n % 4, n % 2)
            n += 1
    for w, kt in enumerate([3, 2, 1, 0]):
        for j, mt in enumerate(range(4)):
            nc.tensor.matmul(pszA[j],
                             lhsT=yT[kt][:, mt * P:(mt + 1) * P],
                             rhs=grX[kt],
                             start=False, stop=(kt == 0))
            if kt == 0:
                drain(pszA[j], mt, FD, False)

    # group B (mt 4..7), banks 4..7
    rhsQ = [grX[0], grX[1], grX[2], grX[3],
            gr[4][:, FD:], gr[5][:, FD:], gr[6][:, FD:], gr[7][:, FD:]]
    mm2_group([4, 5, 6, 7], [4, 5, 6, 7], rhsQ, FD, last_half=False)

    # ---- mm2 half L: out cols 0:512 ----
    # B-mirrors: block (r, c), r < c <= 3 (banks 0..3 free after group A
    # of half Q drained)
    n = 0
    for r in range(4):
        for c in range(r + 1, 4):
            mirror(gr[r], c * P, gr[c], r * P, n % 4, n % 2)
            n += 1
    rhsL = [gr[kt][:, 0:FD] for kt in range(KT)]
    mm2_group([0, 1, 2, 3], [0, 1, 2, 3], rhsL, 0, last_half=False)
    mm2_group([4, 5, 6, 7], [4, 5, 6, 7], rhsL, 0, last_half=True)


def build():
    from contextlib import ExitStack

    nc = bacc.Bacc("TRN2", target_bir_lowering=False, debug=False,
                   num_devices=NCORES)
    xb = nc.dram_tensor("xb", [R, D], F32, kind="ExternalInput").ap()
    out = nc.dram_tensor("out", [R, D], F32, kind="ExternalOutput").ap()
    with tile.TileContext(nc) as tc:
        with ExitStack() as ctx:
            _emit_body(tc, xb, out, ctx)
    nc.compile()
    return nc


_NC_CACHE = {}


def _get_nc():
    if "nc" not in _NC_CACHE:
        _NC_CACHE["nc"] = build()
    return _NC_CACHE["nc"]


def kernel(x: np.ndarray) -> np.ndarray:
    x = np.asarray(x, dtype=np.float32)
    assert x.shape == (N, D), x.shape
    nc = _get_nc()
    in_maps = [{"xb": x[c * R:(c + 1) * R]} for c in range(NCORES)]
    res = run_bass_kernel_spmd(nc, in_maps, list(range(NCORES)))
    return np.concatenate([res.results[c]["out"] for c in range(NCORES)], axis=0)


# revision 37
# speedup vs baseline: 1.2901x; 1.0024x over previous
"""Trainium2 Bass kernel for CosineAttention:

    out = sigmoid((xn @ xn.T) @ x)   where xn = x / ||x_row||

Reassociated: out = sigmoid(diag(sqrt r) . (y @ G)), y = diag(r^1/2) x,
G = y.T @ y  (symmetric, [D, D]); r = 1/||x_row||.  The O(N^2 D)
similarity matrix is never formed, and only ONE scaled fp16 copy of x
is needed (the row-scale sqrt(r) folds into the final sigmoid's
per-partition scale).  G's ~256 diagonal is fp16-safe, so no diagonal
shift machinery is needed.

Sharding: rows of x across 8 cores.  Each core computes the lower
triangle of partial G_c = y_c.T @ y_c (fp32 PSUM), AllReduces the
packed triangle in fp16, reconstructs mirrored blocks by PE transpose,
then computes its [N/8, D] output slice.

Collectives: meshes run serially in doorbell-firing order, and the
first mesh starts only ~25us after the last doorbell (all-ranks
rendezvous + ncfw wake; a pre-fired warmup mesh does NOT absorb this,
it only adds its own mesh time, so none is used).  Payload A = G rows
4..7 lower-tri (832KB, needed first by mm2) is computed and staged
strictly before payload B = G rows 0..3 tri (320KB, needed only by the
last quarter of mm2, so its mesh hides entirely under mm2 compute).
mm2 runs the column-half 512:1024 first (depends only on AR-A), in
PSUM groups of 4 banks so the mirror transposes interleave in the free
banks.  G loads stay off the gpsimd queue so the scheduler cannot
hoist them ahead of AR-B's doorbell.  yT and the G-block mirrors stay
on the PE: DMA-crossbar transposes (dma_start_transpose) measurably
corrupt data when any collective mesh or in-flight DMA shares the
XBAR, and cost ~1.25us of queue occupancy each.
"""

import numpy as np

import concourse.bass as bass  # noqa: F401
import concourse.mybir as mybir
import concourse.tile as tile
from concourse import bacc
from concourse.bass_utils import run_bass_kernel_spmd
from concourse.masks import make_identity

F32 = mybir.dt.float32
BF16 = mybir.dt.bfloat16
F16 = mybir.dt.float16
AFT = mybir.ActivationFunctionType

N, D = 8192, 1024
NCORES = 8
R = N // NCORES  # rows per core
P = 128
RT = R // P      # row tiles per core (8)
KT = D // P      # feature-block tiles (8)
FD = 512         # matmul moving free dim (one PSUM bank of f32)
GROUPS = [list(range(NCORES))]

# AR-A packed layout: [left parts mt=4..7 (4 x 512) | quad parts mt=4..7
# (128,256,384,512)]
A_LEFT = 4 * FD                              # 2048
QW = [(mt - 3) * P for mt in range(4, 8)]    # 128,256,384,512
QOFF = [A_LEFT]
for w in QW[:-1]:
    QOFF.append(QOFF[-1] + w)
TOT_A = A_LEFT + sum(QW)                     # 3328
# AR-B packed layout: rows 0..3 lower-tri widths
BW = [(mt + 1) * P for mt in range(4)]       # 128,256,384,512
BOFF = [0]
for w in BW[:-1]:
    BOFF.append(BOFF[-1] + w)
TOT_B = sum(BW)                              # 1280


def _emit_body(tc, xb, out, ctx):
    nc = tc.nc
    mm_dt = F16
    out_t = out.rearrange("(rt p) d -> rt p d", p=P)

    persist = ctx.enter_context(tc.tile_pool(name="persist", bufs=1))
    load = ctx.enter_context(tc.tile_pool(name="load", bufs=3))
    small = ctx.enter_context(tc.tile_pool(name="small", bufs=1))
    ostage = ctx.enter_context(tc.tile_pool(name="ostage", bufs=8))
    ps = ctx.enter_context(tc.tile_pool(name="ps", bufs=1, space="PSUM"))
    dram = ctx.enter_context(tc.tile_pool(name="dram", bufs=1, space="DRAM"))

    # ---- phase 0: staggered loads on three queues, norms, y ----
    xfall = persist.tile([P, RT, D], F32, tag="xfall")
    xb_r = xb.rearrange("(rt p) d -> p rt d", p=P)
    # tiny dummy sqrt so ACT loads its table while chunk 0 is in flight
    dumin = small.tile([P, 1], F32, tag="dumin")
    dumout = small.tile([P, 1], F32, tag="dumout")
    nc.vector.memset(dumin, 1.0)
    nc.scalar.sqrt(dumout, dumin)
    # tile 0 split in thirds across all three queues so it lands first
    nc.sync.dma_start(out=xfall[:, 0, 0:384], in_=xb_r[:, 0, 0:384])
    nc.gpsimd.dma_start(out=xfall[:, 0, 384:768], in_=xb_r[:, 0, 384:768])
    nc.scalar.dma_start(out=xfall[:, 0, 768:], in_=xb_r[:, 0, 768:])
    ENGS = [nc.sync, nc.gpsimd, nc.scalar, nc.sync, nc.gpsimd,
            nc.scalar, nc.sync]
    for rt in range(1, RT):
        ENGS[rt - 1].dma_start(out=xfall[:, rt, :], in_=xb_r[:, rt, :])

    ss_all = small.tile([P, RT], F32, tag="ss_all")
    iss_all = small.tile([P, RT], F32, tag="iss_all")
    r_all = small.tile([P, RT], F32, tag="r_all")
    sr_all = small.tile([P, RT], F32, tag="sr_all")
    y = []
    for rt in range(RT):
        xf = xfall[:, rt, :]
        sq = load.tile([P, D], BF16, tag="sq")
        nc.vector.affine_mul_reduce(sq, ss_all[:, rt:rt + 1], xf, xf,
                                    1.0, 0.0)
        nc.vector.reciprocal(iss_all[:, rt:rt + 1], ss_all[:, rt:rt + 1])
        nc.scalar.sqrt(r_all[:, rt:rt + 1], iss_all[:, rt:rt + 1])
        nc.scalar.sqrt(sr_all[:, rt:rt + 1], r_all[:, rt:rt + 1])
        t_y = persist.tile([P, D], mm_dt, tag=f"y{rt}", name=f"y{rt}")
        # split across DVE/ACT so neither engine is the phase-0 bottleneck
        nc.vector.tensor_scalar_mul(t_y[:, 0:FD], xf[:, 0:FD],
                                    sr_all[:, rt:rt + 1])
        nc.scalar.activation(out=t_y[:, FD:], in_=xf[:, FD:], func=AFT.Copy,
                             scale=sr_all[:, rt:rt + 1])
        y.append(t_y)

    identb = persist.tile([P, P], mm_dt, tag="identb")
    make_identity(nc, identb)

    # ---- phase 1: mm1.  Pass 1 computes ALL of payload A: A-left
    # (rows 4..7 cols 0:512, banks 4..7) + quad-tri (rows 4..7 cols
    # 512.., banks 0..3); pass 2 computes payload B (rows 0..3 tri).
    # A must be staged strictly before B so A's doorbell fires first.
    psAL = [ps.tile([P, FD], F32, tag=f"acc{4 + i}", name=f"psAL{i}")
            for i in range(4)]
    psQ = [ps.tile([P, QW[i]], F32, tag=f"acc{i}", name=f"psQ{i}")
           for i in range(4)]
    for rt in range(RT):
        for i in range(4):
            nc.tensor.matmul(
                psAL[i],
                lhsT=y[rt][:, (4 + i) * P:(5 + i) * P],
                rhs=y[rt][:, 0:FD],
                start=(rt == 0), stop=(rt == RT - 1),
            )
            nc.tensor.matmul(
                psQ[i],
                lhsT=y[rt][:, (4 + i) * P:(5 + i) * P],
                rhs=y[rt][:, FD:FD + QW[i]],
                start=(rt == 0), stop=(rt == RT - 1),
            )

    # stage payload A on two queues, left parts first
    gA_s = persist.tile([P, TOT_A], mm_dt, tag="gA_s")
    a_in = dram.tile([P, TOT_A], mm_dt, tag="a_in")
    a_out = dram.tile([P, TOT_A], mm_dt, tag="a_out", addr_space="Shared")
    for i in range(4):
        if i % 2 == 0:
            nc.vector.tensor_copy(out=gA_s[:, i * FD:(i + 1) * FD],
                                  in_=psAL[i])
        else:
            nc.scalar.copy(out=gA_s[:, i * FD:(i + 1) * FD], in_=psAL[i])
    nc.sync.dma_start(out=a_in[:, 0:A_LEFT // 2], in_=gA_s[:, 0:A_LEFT // 2])
    nc.scalar.dma_start(out=a_in[:, A_LEFT // 2:A_LEFT],
                        in_=gA_s[:, A_LEFT // 2:A_LEFT])
    for i in range(4):
        if i % 2 == 0:
            nc.vector.tensor_copy(out=gA_s[:, QOFF[i]:QOFF[i] + QW[i]],
                                  in_=psQ[i])
        else:
            nc.scalar.copy(out=gA_s[:, QOFF[i]:QOFF[i] + QW[i]], in_=psQ[i])
    nc.sync.dma_start(out=a_in[:, A_LEFT:A_LEFT + 320],
                      in_=gA_s[:, A_LEFT:A_LEFT + 320])
    nc.scalar.dma_start(out=a_in[:, A_LEFT + 320:A_LEFT + 640],
                        in_=gA_s[:, A_LEFT + 320:A_LEFT + 640])
    nc.sync.dma_start(out=a_in[:, A_LEFT + 640:A_LEFT + 980],
                      in_=gA_s[:, A_LEFT + 640:A_LEFT + 980])
    nc.scalar.dma_start(out=a_in[:, A_LEFT + 980:],
                        in_=gA_s[:, A_LEFT + 980:])

    # pass 2: B-tri (banks 0..3, freed by the quad staging copies)
    psB = [ps.tile([P, BW[mt]], F32, tag=f"acc{mt}", name=f"psB{mt}")
           for mt in range(4)]
    for rt in range(RT):
        for mt in range(4):
            nc.tensor.matmul(
                psB[mt],
                lhsT=y[rt][:, mt * P:(mt + 1) * P],
                rhs=y[rt][:, 0:BW[mt]],
                start=(rt == 0), stop=(rt == RT - 1),
            )
    b_in = dram.tile([P, TOT_B], mm_dt, tag="b_in")
    b_out = dram.tile([P, TOT_B], mm_dt, tag="b_out", addr_space="Shared")
    gB_s = persist.tile([P, TOT_B], mm_dt, tag="gB_s")
    for mt in range(4):
        if mt % 2 == 0:
            nc.vector.tensor_copy(out=gB_s[:, BOFF[mt]:BOFF[mt] + BW[mt]],
                                  in_=psB[mt])
        else:
            nc.scalar.copy(out=gB_s[:, BOFF[mt]:BOFF[mt] + BW[mt]],
                           in_=psB[mt])
    # B staging on the two HWDGE queues (A's staging DMAs are already
    # drained by then), so trigB fires right after its copies land
    nc.sync.dma_start(out=b_in[:, 0:640], in_=gB_s[:, 0:640])
    nc.scalar.dma_start(out=b_in[:, 640:], in_=gB_s[:, 640:])

    # ---- collectives: AR-A first (needed first), AR-B second ----
    nc.gpsimd.collective_compute(
        "AllReduce", mybir.AluOpType.add, replica_groups=GROUPS,
        ins=[a_in.opt()], outs=[a_out.opt()],
    )
    nc.gpsimd.collective_compute(
        "AllReduce", mybir.AluOpType.add, replica_groups=GROUPS,
        ins=[b_in.opt()], outs=[b_out.opt()],
    )

    # ---- hidden in the AR window: yT transposes (4-bank rotation) ----
    yT = []
    for kt in range(KT):
        t_yT = persist.tile([P, D], mm_dt, tag=f"yT{kt}", name=f"yT{kt}")
        for rt in range(RT):
            src = y[rt][:, kt * P:(kt + 1) * P]
            tpt = ps.tile([P, P], mm_dt, tag=f"acc{4 + rt % 4}",
                          name=f"tp{kt}_{rt}")
            nc.tensor.transpose(tpt, src, identb)
            if rt % 2 == 0:
                nc.vector.tensor_copy(out=t_yT[:, rt * P:(rt + 1) * P],
                                      in_=tpt)
            else:
                nc.scalar.copy(out=t_yT[:, rt * P:(rt + 1) * P], in_=tpt)
        yT.append(t_yT)

    # preload the Sigmoid table while the mesh runs (input reads the
    # staged payload so the scheduler cannot hoist it into phase 0,
    # where it would thrash the ACT table against the Sqrts)
    dsin = small.tile([P, 1], F32, tag="dsin")
    nc.scalar.activation(out=dsin, in_=gA_s[:, 0:1], func=AFT.Sigmoid)

    # ---- G loads on sync/scalar ONLY (a gpsimd G-load can be hoisted
    # ahead of AR-B's doorbell on that queue, delaying the whole train)
    gr = [persist.tile([P, D], mm_dt, tag=f"gr{kt}", name=f"gr{kt}")
          for kt in range(KT)]
    # wave order is kt 7..4, and the early mirrors source only quad
    # regions, so load quads descending first, then the left parts
    # (first needed by the cross-mirrors a few us later)
    qe = [nc.sync, nc.scalar]
    for j, i in enumerate([3, 2, 1, 0]):
        qe[j % 2].dma_start(out=gr[4 + i][:, FD:FD + QW[i]],
                            in_=a_out[:, QOFF[i]:QOFF[i] + QW[i]])
    for j, i in enumerate([3, 2, 1, 0]):
        qe[j % 2].dma_start(out=gr[4 + i][:, 0:FD],
                            in_=a_out[:, i * FD:(i + 1) * FD])
    # B loads (fire when AR-B lands; gpsimd queue, after AR-B's trigger)
    for mt in range(4):
        nc.gpsimd.dma_start(out=gr[mt][:, 0:BW[mt]],
                            in_=b_out[:, BOFF[mt]:BOFF[mt] + BW[mt]])

    mir_n = [0]

    def mirror(dst_tile, dst_lo, src_tile, src_lo, slot, odd):
        mir_n[0] += 1
        tpu = ps.tile([P, P], mm_dt, tag=f"acc{slot}",
                      name=f"mir{mir_n[0]}")
        nc.tensor.transpose(tpu, src_tile[:, src_lo:src_lo + P], identb)
        if odd:
            nc.scalar.copy(out=dst_tile[:, dst_lo:dst_lo + P], in_=tpu)
        else:
            nc.vector.tensor_copy(out=dst_tile[:, dst_lo:dst_lo + P],
                                  in_=tpu)

    sig_n = [0]

    def drain(psz, mt, lo, last_half):
        # sigmoid with per-partition scale sqrt(r); fp16 out via SWDGE
        # (casts to f32), except f32 + HWDGE half-stores for the final
        # tiles to shorten the terminal drain.
        i = sig_n[0]
        sig_n[0] += 1
        if last_half and mt >= 6:
            # final two tiles: f32 sigmoid + half-stores on both HWDGE
            # queues to shorten the terminal drain
            ob32 = ostage.tile([P, FD], F32, tag="ob32")
            nc.scalar.activation(out=ob32, in_=psz, func=AFT.Sigmoid,
                                 scale=sr_all[:, mt:mt + 1])
            nc.sync.dma_start(out=out_t[mt][:, lo:lo + FD // 2],
                              in_=ob32[:, 0:FD // 2])
            nc.scalar.dma_start(out=out_t[mt][:, lo + FD // 2:lo + FD],
                                in_=ob32[:, FD // 2:])
        elif last_half and mt % 2 == 1:
            ob32 = ostage.tile([P, FD], F32, tag="ob32")
            nc.scalar.activation(out=ob32, in_=psz, func=AFT.Sigmoid,
                                 scale=sr_all[:, mt:mt + 1])
            nc.sync.dma_start(out=out_t[mt][:, lo:lo + FD], in_=ob32)
        else:
            ob = ostage.tile([P, FD], mm_dt, tag="ob")
            nc.scalar.activation(out=ob, in_=psz, func=AFT.Sigmoid,
                                 scale=sr_all[:, mt:mt + 1])
            nc.gpsimd.dma_start(out=out_t[mt][:, lo:lo + FD], in_=ob)

    def mm2_group(mts, banks, rhs_list, lo, last_half=False):
        psz = {}
        for j, mt in enumerate(mts):
            psz[mt] = ps.tile([P, FD], F32, tag=f"acc{banks[j]}",
                              name=f"psz{lo}_{mt}_{banks[j]}")
        for w, kt in enumerate([7, 6, 5, 4, 3, 2, 1]):
            for mt in mts:
                nc.tensor.matmul(
                    psz[mt],
                    lhsT=yT[kt][:, mt * P:(mt + 1) * P],
                    rhs=rhs_list[kt],
                    start=(w == 0), stop=False,
                )
        for mt in mts:
            nc.tensor.matmul(
                psz[mt],
                lhsT=yT[0][:, mt * P:(mt + 1) * P],
                rhs=rhs_list[0],
                start=False, stop=True,
            )
            drain(psz[mt], mt, lo, last_half)

    # group A (mt 0..3, banks 0..3) waves kt 7..4 start right away; the
    # 16 cross-mirrors (rows 0..3 cols 512:1024 = T of rows 4..7 cols
    # 0:512) run in banks 4..7 while group A holds 0..3.
    grX = [persist.tile([P, FD], mm_dt, tag=f"grX{r}", name=f"grX{r}")
           for r in range(4)]

    pszA = [ps.tile([P, FD], F32, tag=f"acc{j}", name=f"pszqA{j}")
            for j in range(4)]
    # A-mirrors (block (r, c), 4 <= r < c, into banks 4..7) interleave
    # with the kt 7..4 waves: wave kt needs only mirrors in column
    # blocks > kt of row kt, so wave 7 starts as soon as gr[7] loads
    n = 0
    AMIR = {7: [(6, 7)], 6: [(5, 6), (5, 7)], 5: [(4, 5), (4, 6), (4, 7)]}
    for w, kt in enumerate([7, 6, 5, 4]):
        for j, mt in enumerate(range(4)):
            nc.tensor.matmul(pszA[j],
                             lhsT=yT[kt][:, mt * P:(mt + 1) * P],
                             rhs=gr[kt][:, FD:],
                             start=(w == 0), stop=False)
        for r, c in AMIR.get(kt, []):
            mirror(gr[r], c * P, gr[c], r * P, 4 + n % 4, n % 2)
            n += 1
    n = 0
    for r in range(3, -1, -1):
        for c in range(4, 8):
            mirror(grX[r], (c - 4) * P, gr[c], r * P, 4 + n % 4, n % 2)
            n += 1
    for w, kt in enumerate([3, 2, 1, 0]):
        for j, mt in enumerate(range(4)):
            nc.tensor.matmul(pszA[j],
                             lhsT=yT[kt][:, mt * P:(mt + 1) * P],
                             rhs=grX[kt],
                             start=False, stop=(kt == 0))
            if kt == 0:
                drain(pszA[j], mt, FD, False)

    # group B (mt 4..7), banks 4..7
    rhsQ = [grX[0], grX[1], grX[2], grX[3],
            gr[4][:, FD:], gr[5][:, FD:], gr[6][:, FD:], gr[7][:, FD:]]
    mm2_group([4, 5, 6, 7], [4, 5, 6, 7], rhsQ, FD, last_half=False)

    # ---- mm2 half L: out cols 0:512 ----
    # B-mirrors: block (r, c), r < c <= 3 (banks 0..3 free after group A
    # of half Q drained)
    n = 0
    for r in range(4):
        for c in range(r + 1, 4):
            mirror(gr[r], c * P, gr[c], r * P, n % 4, n % 2)
            n += 1
    rhsL = [gr[kt][:, 0:FD] for kt in range(KT)]
    mm2_group([0, 1, 2, 3], [0, 1, 2, 3], rhsL, 0, last_half=False)
    mm2_group([4, 5, 6, 7], [4, 5, 6, 7], rhsL, 0, last_half=True)


def build():
    from contextlib import ExitStack

    nc = bacc.Bacc("TRN2", target_bir_lowering=False, debug=False,
                   num_devices=NCORES)
    xb = nc.dram_tensor("xb", [R, D], F32, kind="ExternalInput").ap()
    out = nc.dram_tensor("out", [R, D], F32, kind="ExternalOutput").ap()
    with tile.TileContext(nc) as tc:
        with ExitStack() as ctx:
            _emit_body(tc, xb, out, ctx)
    nc.compile()
    return nc


_NC_CACHE = {}


def _get_nc():
    if "nc" not in _NC_CACHE:
        _NC_CACHE["nc"] = build()
    return _NC_CACHE["nc"]


def kernel(x: np.ndarray) -> np.ndarray:
    x = np.asarray(x, dtype=np.float32)
    assert x.shape == (N, D), x.shape
    nc = _get_nc()
    in_maps = [{"xb": x[c * R:(c + 1) * R]} for c in range(NCORES)]
    res = run_bass_kernel_spmd(nc, in_maps, list(range(NCORES)))
    return np.concatenate([res.results[c]["out"] for c in range(NCORES)], axis=0)


# revision 39
# speedup vs baseline: 1.3847x; 1.0732x over previous
"""Trainium2 Bass kernel for CosineAttention:

    out = sigmoid((xn @ xn.T) @ x)   where xn = x / ||x_row||

Reassociated: out = sigmoid(diag(sqrt r) . (y @ G)), y = diag(r^1/2) x,
G = y.T @ y  (symmetric, [D, D]); r = 1/||x_row||.  The O(N^2 D)
similarity matrix is never formed, and only ONE scaled fp16 copy of x
is needed (the row-scale sqrt(r) folds into the final sigmoid's
per-partition scale).  G's ~256 diagonal is fp16-safe, so no diagonal
shift machinery is needed.

Sharding: rows of x across 8 cores.  Each core computes the lower
triangle of partial G_c = y_c.T @ y_c (fp32 PSUM), AllReduces the
packed triangle in fp16, reconstructs mirrored blocks by PE transpose,
then computes its [N/8, D] output slice.

Collectives: meshes run serially in doorbell-firing order, and the
first mesh starts only ~25us after the last doorbell (all-ranks
rendezvous + ncfw wake; a pre-fired warmup mesh does NOT absorb this,
it only adds its own mesh time, so none is used).  Payload A = G rows
4..7 lower-tri (832KB, needed first by mm2) is computed and staged
strictly before payload B = G rows 0..3 tri (320KB, needed only by the
last quarter of mm2, so its mesh hides entirely under mm2 compute).
mm2 runs the column-half 512:1024 first (depends only on AR-A), in
PSUM groups of 4 banks so the mirror transposes interleave in the free
banks.  G loads stay off the gpsimd queue so the scheduler cannot
hoist them ahead of AR-B's doorbell.  yT and the G-block mirrors stay
on the PE: DMA-crossbar transposes (dma_start_transpose) measurably
corrupt data when any collective mesh or in-flight DMA shares the
XBAR, and cost ~1.25us of queue occupancy each.
"""

import numpy as np

import concourse.bass as bass  # noqa: F401
import concourse.mybir as mybir
import concourse.tile as tile
from concourse import bacc
from concourse.bass_utils import run_bass_kernel_spmd
from concourse.masks import make_identity

F32 = mybir.dt.float32
BF16 = mybir.dt.bfloat16
F16 = mybir.dt.float16
AFT = mybir.ActivationFunctionType

N, D = 8192, 1024
NCORES = 8
R = N // NCORES  # rows per core
P = 128
RT = R // P      # row tiles per core (8)
KT = D // P      # feature-block tiles (8)
FD = 512         # matmul moving free dim (one PSUM bank of f32)
GROUPS = [list(range(NCORES))]

# AR-A packed layout: [left parts mt=4..7 (4 x 512) | quad parts mt=4..7
# (128,256,384,512)]
A_LEFT = 4 * FD                              # 2048
QW = [(mt - 3) * P for mt in range(4, 8)]    # 128,256,384,512
QOFF = [A_LEFT]
for w in QW[:-1]:
    QOFF.append(QOFF[-1] + w)
TOT_A = A_LEFT + sum(QW)                     # 3328
# AR-B packed layout: rows 0..3 lower-tri widths
BW = [(mt + 1) * P for mt in range(4)]       # 128,256,384,512
BOFF = [0]
for w in BW[:-1]:
    BOFF.append(BOFF[-1] + w)
TOT_B = sum(BW)                              # 1280


def _emit_body(tc, xb, out, ctx):
    nc = tc.nc
    mm_dt = F16
    out_t = out.rearrange("(rt p) d -> rt p d", p=P)

    persist = ctx.enter_context(tc.tile_pool(name="persist", bufs=1))
    load = ctx.enter_context(tc.tile_pool(name="load", bufs=3))
    small = ctx.enter_context(tc.tile_pool(name="small", bufs=1))
    ostage = ctx.enter_context(tc.tile_pool(name="ostage", bufs=8))
    ps = ctx.enter_context(tc.tile_pool(name="ps", bufs=1, space="PSUM"))
    dram = ctx.enter_context(tc.tile_pool(name="dram", bufs=1, space="DRAM"))

    # ---- phase 0: staggered loads on three queues, norms, y ----
    xfall = persist.tile([P, RT, D], F32, tag="xfall")
    xb_r = xb.rearrange("(rt p) d -> p rt d", p=P)
    # tiny dummy sqrt so ACT loads its table while chunk 0 is in flight
    dumin = small.tile([P, 1], F32, tag="dumin")
    dumout = small.tile([P, 1], F32, tag="dumout")
    nc.vector.memset(dumin, 1.0)
    nc.scalar.sqrt(dumout, dumin)
    # tile 0 split in thirds across all three queues so it lands first
    nc.sync.dma_start(out=xfall[:, 0, 0:384], in_=xb_r[:, 0, 0:384])
    nc.gpsimd.dma_start(out=xfall[:, 0, 384:768], in_=xb_r[:, 0, 384:768])
    nc.scalar.dma_start(out=xfall[:, 0, 768:], in_=xb_r[:, 0, 768:])
    ENGS = [nc.sync, nc.gpsimd, nc.scalar, nc.sync, nc.gpsimd,
            nc.scalar, nc.sync]
    for rt in range(1, RT):
        ENGS[rt - 1].dma_start(out=xfall[:, rt, :], in_=xb_r[:, rt, :])

    ss_all = small.tile([P, RT], F32, tag="ss_all")
    iss_all = small.tile([P, RT], F32, tag="iss_all")
    r_all = small.tile([P, RT], F32, tag="r_all")
    sr_all = small.tile([P, RT], F32, tag="sr_all")
    y = []
    for rt in range(RT):
        xf = xfall[:, rt, :]
        sq = load.tile([P, D], BF16, tag="sq")
        nc.vector.affine_mul_reduce(sq, ss_all[:, rt:rt + 1], xf, xf,
                                    1.0, 0.0)
        nc.vector.reciprocal(iss_all[:, rt:rt + 1], ss_all[:, rt:rt + 1])
        nc.scalar.sqrt(r_all[:, rt:rt + 1], iss_all[:, rt:rt + 1])
        nc.scalar.sqrt(sr_all[:, rt:rt + 1], r_all[:, rt:rt + 1])
        t_y = persist.tile([P, D], mm_dt, tag=f"y{rt}", name=f"y{rt}")
        # split across DVE/ACT so neither engine is the phase-0 bottleneck
        nc.vector.tensor_scalar_mul(t_y[:, 0:FD], xf[:, 0:FD],
                                    sr_all[:, rt:rt + 1])
        nc.scalar.activation(out=t_y[:, FD:], in_=xf[:, FD:], func=AFT.Copy,
                             scale=sr_all[:, rt:rt + 1])
        y.append(t_y)

    identb = persist.tile([P, P], mm_dt, tag="identb")
    make_identity(nc, identb)

    # ---- phase 1: mm1.  Pass 1 computes ALL of payload A: A-left
    # (rows 4..7 cols 0:512, banks 4..7) + quad-tri (rows 4..7 cols
    # 512.., banks 0..3); pass 2 computes payload B (rows 0..3 tri).
    # A must be staged strictly before B so A's doorbell fires first.
    psAL = [ps.tile([P, FD], F32, tag=f"acc{4 + i}", name=f"psAL{i}")
            for i in range(4)]
    psQ = [ps.tile([P, QW[i]], F32, tag=f"acc{i}", name=f"psQ{i}")
           for i in range(4)]
    for rt in range(RT):
        for i in range(4):
            nc.tensor.matmul(
                psAL[i],
                lhsT=y[rt][:, (4 + i) * P:(5 + i) * P],
                rhs=y[rt][:, 0:FD],
                start=(rt == 0), stop=(rt == RT - 1),
            )
            nc.tensor.matmul(
                psQ[i],
                lhsT=y[rt][:, (4 + i) * P:(5 + i) * P],
                rhs=y[rt][:, FD:FD + QW[i]],
                start=(rt == 0), stop=(rt == RT - 1),
            )

    # stage payload A on two queues, left parts first
    gA_s = persist.tile([P, TOT_A], mm_dt, tag="gA_s")
    a_in = dram.tile([P, TOT_A], mm_dt, tag="a_in")
    a_out = dram.tile([P, TOT_A], mm_dt, tag="a_out", addr_space="Shared")
    for i in range(4):
        if i % 2 == 0:
            nc.vector.tensor_copy(out=gA_s[:, i * FD:(i + 1) * FD],
                                  in_=psAL[i])
        else:
            nc.scalar.copy(out=gA_s[:, i * FD:(i + 1) * FD], in_=psAL[i])
    nc.sync.dma_start(out=a_in[:, 0:A_LEFT // 2], in_=gA_s[:, 0:A_LEFT // 2])
    nc.scalar.dma_start(out=a_in[:, A_LEFT // 2:A_LEFT],
                        in_=gA_s[:, A_LEFT // 2:A_LEFT])
    for i in range(4):
        if i % 2 == 0:
            nc.vector.tensor_copy(out=gA_s[:, QOFF[i]:QOFF[i] + QW[i]],
                                  in_=psQ[i])
        else:
            nc.scalar.copy(out=gA_s[:, QOFF[i]:QOFF[i] + QW[i]], in_=psQ[i])
    nc.sync.dma_start(out=a_in[:, A_LEFT:A_LEFT + 320],
                      in_=gA_s[:, A_LEFT:A_LEFT + 320])
    nc.scalar.dma_start(out=a_in[:, A_LEFT + 320:A_LEFT + 640],
                        in_=gA_s[:, A_LEFT + 320:A_LEFT + 640])
    nc.sync.dma_start(out=a_in[:, A_LEFT + 640:A_LEFT + 980],
                      in_=gA_s[:, A_LEFT + 640:A_LEFT + 980])
    nc.scalar.dma_start(out=a_in[:, A_LEFT + 980:],
                        in_=gA_s[:, A_LEFT + 980:])

    # pass 2: B-tri (banks 0..3, freed by the quad staging copies)
    psB = [ps.tile([P, BW[mt]], F32, tag=f"acc{mt}", name=f"psB{mt}")
           for mt in range(4)]
    for rt in range(RT):
        for mt in range(4):
            nc.tensor.matmul(
                psB[mt],
                lhsT=y[rt][:, mt * P:(mt + 1) * P],
                rhs=y[rt][:, 0:BW[mt]],
                start=(rt == 0), stop=(rt == RT - 1),
            )
    b_in = dram.tile([P, TOT_B], mm_dt, tag="b_in")
    b_out = dram.tile([P, TOT_B], mm_dt, tag="b_out", addr_space="Shared")
    gB_s = persist.tile([P, TOT_B], mm_dt, tag="gB_s")
    for mt in range(4):
        if mt % 2 == 0:
            nc.vector.tensor_copy(out=gB_s[:, BOFF[mt]:BOFF[mt] + BW[mt]],
                                  in_=psB[mt])
        else:
            nc.scalar.copy(out=gB_s[:, BOFF[mt]:BOFF[mt] + BW[mt]],
                           in_=psB[mt])
    # ---- collective AR-A (AR-B is emitted after the G loads: its
    # staging is deliberately chained behind AR-A's completion so its
    # doorbell fires only after the mesh train has started -- the first
    # mesh launches ~26us after the LAST-fired doorbell, so letting
    # trigB fire early would gate the whole train on pass 2 + staging;
    # fired late, AR-B still runs ~1.3us after its own doorbell)
    nc.gpsimd.collective_compute(
        "AllReduce", mybir.AluOpType.add, replica_groups=GROUPS,
        ins=[a_in.opt()], outs=[a_out.opt()],
    )

    # ---- hidden in the AR window: yT transposes (4-bank rotation) ----
    yT = []
    for kt in range(KT):
        t_yT = persist.tile([P, D], mm_dt, tag=f"yT{kt}", name=f"yT{kt}")
        for rt in range(RT):
            src = y[rt][:, kt * P:(kt + 1) * P]
            tpt = ps.tile([P, P], mm_dt, tag=f"acc{4 + rt % 4}",
                          name=f"tp{kt}_{rt}")
            nc.tensor.transpose(tpt, src, identb)
            if rt % 2 == 0:
                nc.vector.tensor_copy(out=t_yT[:, rt * P:(rt + 1) * P],
                                      in_=tpt)
            else:
                nc.scalar.copy(out=t_yT[:, rt * P:(rt + 1) * P], in_=tpt)
        yT.append(t_yT)

    # preload the Sigmoid table while the mesh runs (input reads the
    # staged payload so the scheduler cannot hoist it into phase 0,
    # where it would thrash the ACT table against the Sqrts)
    dsin = small.tile([P, 1], F32, tag="dsin")
    nc.scalar.activation(out=dsin, in_=gA_s[:, 0:1], func=AFT.Sigmoid)

    # ---- G loads on sync/scalar ONLY (a gpsimd G-load can be hoisted
    # ahead of AR-B's doorbell on that queue, delaying the whole train)
    gr = [persist.tile([P, D], mm_dt, tag=f"gr{kt}", name=f"gr{kt}")
          for kt in range(KT)]
    # wave order is kt 7..4, and the early mirrors source only quad
    # regions, so load quads descending first, then the left parts
    # (first needed by the cross-mirrors a few us later)
    qe = [nc.sync, nc.scalar]
    for j, i in enumerate([3, 2, 1, 0]):
        qe[j % 2].dma_start(out=gr[4 + i][:, FD:FD + QW[i]],
                            in_=a_out[:, QOFF[i]:QOFF[i] + QW[i]])
    for j, i in enumerate([3, 2, 1, 0]):
        qe[j % 2].dma_start(out=gr[4 + i][:, 0:FD],
                            in_=a_out[:, i * FD:(i + 1) * FD])
    # delay AR-B's staging behind AR-A's completion: an idempotent add
    # of an AR-A-derived zero to the payload's first column makes the
    # staging DMAs (and hence trigB) wait for the gr[7] load
    zt = small.tile([P, 1], mm_dt, tag="zt")
    nc.vector.tensor_scalar_mul(zt, gr[7][:, 0:1], 0.0)
    nc.vector.tensor_add(gB_s[:, 0:1], gB_s[:, 0:1], zt)
    nc.sync.dma_start(out=b_in[:, 0:640], in_=gB_s[:, 0:640])
    nc.scalar.dma_start(out=b_in[:, 640:], in_=gB_s[:, 640:])
    nc.gpsimd.collective_compute(
        "AllReduce", mybir.AluOpType.add, replica_groups=GROUPS,
        ins=[b_in.opt()], outs=[b_out.opt()],
    )
    # B loads (fire when AR-B lands; gpsimd queue, after AR-B's trigger)
    for mt in range(4):
        nc.gpsimd.dma_start(out=gr[mt][:, 0:BW[mt]],
                            in_=b_out[:, BOFF[mt]:BOFF[mt] + BW[mt]])

    mir_n = [0]

    def mirror(dst_tile, dst_lo, src_tile, src_lo, slot, odd):
        mir_n[0] += 1
        tpu = ps.tile([P, P], mm_dt, tag=f"acc{slot}",
                      name=f"mir{mir_n[0]}")
        nc.tensor.transpose(tpu, src_tile[:, src_lo:src_lo + P], identb)
        if odd:
            nc.scalar.copy(out=dst_tile[:, dst_lo:dst_lo + P], in_=tpu)
        else:
            nc.vector.tensor_copy(out=dst_tile[:, dst_lo:dst_lo + P],
                                  in_=tpu)

    sig_n = [0]

    def drain(psz, mt, lo, last_half):
        # sigmoid with per-partition scale sqrt(r); fp16 out via SWDGE
        # (casts to f32), except f32 + HWDGE half-stores for the final
        # tiles to shorten the terminal drain.
        i = sig_n[0]
        sig_n[0] += 1
        if last_half and mt >= 6:
            # final two tiles: f32 sigmoid + half-stores on both HWDGE
            # queues to shorten the terminal drain
            ob32 = ostage.tile([P, FD], F32, tag="ob32")
            nc.scalar.activation(out=ob32, in_=psz, func=AFT.Sigmoid,
                                 scale=sr_all[:, mt:mt + 1])
            nc.sync.dma_start(out=out_t[mt][:, lo:lo + FD // 2],
                              in_=ob32[:, 0:FD // 2])
            nc.scalar.dma_start(out=out_t[mt][:, lo + FD // 2:lo + FD],
                                in_=ob32[:, FD // 2:])
        elif last_half and mt % 2 == 1:
            ob32 = ostage.tile([P, FD], F32, tag="ob32")
            nc.scalar.activation(out=ob32, in_=psz, func=AFT.Sigmoid,
                                 scale=sr_all[:, mt:mt + 1])
            nc.sync.dma_start(out=out_t[mt][:, lo:lo + FD], in_=ob32)
        else:
            ob = ostage.tile([P, FD], mm_dt, tag="ob")
            nc.scalar.activation(out=ob, in_=psz, func=AFT.Sigmoid,
                                 scale=sr_all[:, mt:mt + 1])
            nc.gpsimd.dma_start(out=out_t[mt][:, lo:lo + FD], in_=ob)

    def mm2_group(mts, banks, rhs_list, lo, last_half=False):
        psz = {}
        for j, mt in enumerate(mts):
            psz[mt] = ps.tile([P, FD], F32, tag=f"acc{banks[j]}",
                              name=f"psz{lo}_{mt}_{banks[j]}")
        for w, kt in enumerate([7, 6, 5, 4, 3, 2, 1]):
            for mt in mts:
                nc.tensor.matmul(
                    psz[mt],
                    lhsT=yT[kt][:, mt * P:(mt + 1) * P],
                    rhs=rhs_list[kt],
                    start=(w == 0), stop=False,
                )
        for mt in mts:
            nc.tensor.matmul(
                psz[mt],
                lhsT=yT[0][:, mt * P:(mt + 1) * P],
                rhs=rhs_list[0],
                start=False, stop=True,
            )
            drain(psz[mt], mt, lo, last_half)

    # group A (mt 0..3, banks 0..3) waves kt 7..4 start right away; the
    # 16 cross-mirrors (rows 0..3 cols 512:1024 = T of rows 4..7 cols
    # 0:512) run in banks 4..7 while group A holds 0..3.
    grX = [persist.tile([P, FD], mm_dt, tag=f"grX{r}", name=f"grX{r}")
           for r in range(4)]

    pszA = [ps.tile([P, FD], F32, tag=f"acc{j}", name=f"pszqA{j}")
            for j in range(4)]
    # A-mirrors (block (r, c), 4 <= r < c, into banks 4..7) interleave
    # with the kt 7..4 waves: wave kt needs only mirrors in column
    # blocks > kt of row kt, so wave 7 starts as soon as gr[7] loads
    n = 0
    AMIR = {7: [(6, 7)], 6: [(5, 6), (5, 7)], 5: [(4, 5), (4, 6), (4, 7)]}
    for w, kt in enumerate([7, 6, 5, 4]):
        for j, mt in enumerate(range(4)):
            nc.tensor.matmul(pszA[j],
                             lhsT=yT[kt][:, mt * P:(mt + 1) * P],
                             rhs=gr[kt][:, FD:],
                             start=(w == 0), stop=False)
        for r, c in AMIR.get(kt, []):
            mirror(gr[r], c * P, gr[c], r * P, 4 + n % 4, n % 2)
            n += 1
    n = 0
    for r in range(3, -1, -1):
        for c in range(4, 8):
            mirror(grX[r], (c - 4) * P, gr[c], r * P, 4 + n % 4, n % 2)
            n += 1
    for w, kt in enumerate([3, 2, 1, 0]):
        for j, mt in enumerate(range(4)):
            nc.tensor.matmul(pszA[j],
                             lhsT=yT[kt][:, mt * P:(mt + 1) * P],
                             rhs=grX[kt],
                             start=False, stop=(kt == 0))
            if kt == 0:
                drain(pszA[j], mt, FD, False)

    # group B (mt 4..7), banks 4..7
    rhsQ = [grX[0], grX[1], grX[2], grX[3],
            gr[4][:, FD:], gr[5][:, FD:], gr[6][:, FD:], gr[7][:, FD:]]
    mm2_group([4, 5, 6, 7], [4, 5, 6, 7], rhsQ, FD, last_half=False)

    # ---- mm2 half L: out cols 0:512 ----
    # B-mirrors: block (r, c), r < c <= 3 (banks 0..3 free after group A
    # of half Q drained)
    n = 0
    for r in range(4):
        for c in range(r + 1, 4):
            mirror(gr[r], c * P, gr[c], r * P, n % 4, n % 2)
            n += 1
    rhsL = [gr[kt][:, 0:FD] for kt in range(KT)]
    mm2_group([0, 1, 2, 3], [0, 1, 2, 3], rhsL, 0, last_half=False)
    mm2_group([4, 5, 6, 7], [4, 5, 6, 7], rhsL, 0, last_half=True)


def build():
    from contextlib import ExitStack

    nc = bacc.Bacc("TRN2", target_bir_lowering=False, debug=False,
                   num_devices=NCORES)
    xb = nc.dram_tensor("xb", [R, D], F32, kind="ExternalInput").ap()
    out = nc.dram_tensor("out", [R, D], F32, kind="ExternalOutput").ap()
    with tile.TileContext(nc) as tc:
        with ExitStack() as ctx:
            _emit_body(tc, xb, out, ctx)
    nc.compile()
    return nc


_NC_CACHE = {}


def _get_nc():
    if "nc" not in _NC_CACHE:
        _NC_CACHE["nc"] = build()
    return _NC_CACHE["nc"]


def kernel(x: np.ndarray) -> np.ndarray:
    x = np.asarray(x, dtype=np.float32)
    assert x.shape == (N, D), x.shape
    nc = _get_nc()
    in_maps = [{"xb": x[c * R:(c + 1) * R]} for c in range(NCORES)]
    res = run_bass_kernel_spmd(nc, in_maps, list(range(NCORES)))
    return np.concatenate([res.results[c]["out"] for c in range(NCORES)], axis=0)
